# revision 16
# baseline (speedup 1.0000x reference)
"""Trainium2 Bass kernel for causal self-attention with RoPE.

Sharding: 8 cores = 2 batches x 4 head-groups (4 heads each).
Each core computes its batch's qkv projection for its heads, RoPE,
causal flash-attention, and a partial output projection; the host sums
the 4 partials per batch.

All matmuls run as fp32r (full-rate) except scores, whose operands are
bf16 (produced by the RoPE pass). Softmax uses no max-subtraction
(scores are O(5) bounded), and the denominator comes from an extra
ones-column in the PV stationary operand.
"""

import os

import numpy as np

NUM_HEADS = 16
B, T, C = 2, 2048, 1024
D = C // NUM_HEADS  # 64
HPC = 4             # heads per core
NCORES = 8

_CACHE = {}

LAST_EXEC_NS = None
LAST_RESULTS = None


def _build_body(nc, reps=1):
    import concourse.bass as bass
    import concourse.mybir as mybir
    import concourse.tile as tile
    from contextlib import ExitStack

    F32 = mybir.dt.float32
    F32R = mybir.dt.float32r
    BF16 = mybir.dt.bfloat16
    AF = mybir.ActivationFunctionType

    xT = nc.dram_tensor("xT", [C, T], BF16, kind="ExternalInput").ap()
    wT = nc.dram_tensor("wT", [C, 768], BF16, kind="ExternalInput").ap()
    projT = nc.dram_tensor("projT", [256, C], BF16, kind="ExternalInput").ap()
    CS = nc.dram_tensor("CS", [128, T], BF16, kind="ExternalInput").ap()
    SN = nc.dram_tensor("SN", [128, T], BF16, kind="ExternalInput").ap()
    maskc = nc.dram_tensor("maskc", [128, 128], BF16, kind="ExternalInput").ap()
    identb = nc.dram_tensor("identb", [128, 128], BF16, kind="ExternalInput").ap()
    ident = nc.dram_tensor("ident", [128, 128], F32, kind="ExternalInput").ap()
    out = nc.dram_tensor("out", [T, C], BF16, kind="ExternalOutput").ap()
    linv_dram = nc.dram_tensor("linv_scratch", [64, 128], F32).ap()

    with tile.TileContext(nc) as tc, ExitStack() as ctx:
        singles = ctx.enter_context(tc.tile_pool(name="singles", bufs=1))
        stream = ctx.enter_context(tc.tile_pool(name="stream", bufs=4))
        ptiles = ctx.enter_context(tc.tile_pool(name="ptiles", bufs=3))

        # weights first so the first qkv matmul's inputs arrive early;
        # rope tables / proj weights are needed later and load behind them
        w_sb = singles.tile([128, 8, 768], BF16)
        for ci in range(8):
            nc.gpsimd.dma_start(
                out=w_sb[:, ci, :], in_=wT[ci * 128:(ci + 1) * 128, :]
            )
        mask_sb = singles.tile([128, 128], BF16)
        nc.gpsimd.dma_start(out=mask_sb[:], in_=maskc)
        idb_sb = singles.tile([128, 128], BF16)
        nc.gpsimd.dma_start(out=idb_sb[:], in_=identb)
        id_sb = singles.tile([128, 128], F32)
        nc.gpsimd.dma_start(out=id_sb[:], in_=ident)
        cs_sb = singles.tile([128, T], BF16)
        sn_sb = singles.tile([128, T], BF16)
        pj_sb = singles.tile([128, 2, C], BF16)

        q_rot = singles.tile([128, 2, T], BF16)
        k_rot = singles.tile([128, 2, T], BF16)
        v_sb = singles.tile([128, 16, 65 * HPC], BF16)
        # per-(head, q-block) attention outputs at partitions 0-64
        # (row 64 = softmax denominator); block index r = h*4 + qb
        u_sb = singles.tile([65, 16, 512], F32)
        u2_sb = singles.tile([128, 2, T], BF16)
        # softmax denominators: q-block qb at partitions 32qb..32qb+3
        l_sb = singles.tile([128, 512], F32)
        linv_col = singles.tile([128, 64], F32)

        # ones columns interleaved with v (col 64 of every 65-col head block)
        v_h = v_sb[:].rearrange("p t (h c) -> p t h c", c=65)
        nc.vector.memset(v_h[:, :, :, 64:65], 1.0)

        for rep in range(reps):
            # ---- Phase 1: QKV projection + RoPE ----
            # 256-wide psum sub-chunks so qkv accumulators double-buffer in
            # 6 banks and RoPE evacuation never stalls the PE.
            with tc.tile_pool(name=f"qkvps{rep}", bufs=2, space="PSUM") as qkvps:
                for ch in range(4):
                    tok = slice(ch * 512, (ch + 1) * 512)
                    q_ps = [qkvps.tile([128, 512], F32, tag="qps", name="qps")
                            for _ in range(2)]
                    k_ps = [qkvps.tile([128, 512], F32, tag="kps", name="kps")
                            for _ in range(2)]
                    v_ps = [qkvps.tile([128, 512], F32, tag="vps", name="vps")
                            for _ in range(2)]
                    for ci in range(8):
                        xt = stream.tile([128, 512], BF16, tag="xt", name="xt")
                        nc.sync.dma_start(
                            out=xt[:], in_=xT[ci * 128:(ci + 1) * 128, tok]
                        )
                        for sub in range(2):
                            for ft in range(2):
                                nc.tensor.matmul(
                                    q_ps[sub][:, ft * 256:(ft + 1) * 256],
                                    w_sb[:, ci, ft * 128:(ft + 1) * 128],
                                    xt[:, sub * 256:(sub + 1) * 256],
                                    start=(ci == 0 and ft == 0),
                                    stop=(ci == 7 and ft == 1),
                                )
                                nc.tensor.matmul(
                                    k_ps[sub][:, ft * 256:(ft + 1) * 256],
                                    w_sb[:, ci, 256 + ft * 128:256 + (ft + 1) * 128],
                                    xt[:, sub * 256:(sub + 1) * 256],
                                    start=(ci == 0 and ft == 0),
                                    stop=(ci == 7 and ft == 1),
                                )
                            for ts in range(2):
                                nc.tensor.matmul(
                                    v_ps[sub][:, ts * 256:(ts + 1) * 256],
                                    xt[:, sub * 256 + ts * 128:sub * 256 + (ts + 1) * 128],
                                    w_sb[:, ci, 512:768],
                                    start=(ci == 0 and ts == 0),
                                    stop=(ci == 7 and ts == 1),
                                )
                    if ch == 0:
                        # behind the first chunk's x tiles; needed by RoPE
                        nc.gpsimd.dma_start(out=cs_sb[:], in_=CS)
                        nc.gpsimd.dma_start(out=sn_sb[:], in_=SN)
                    # Evacuate all qkv psum banks through the (otherwise idle)
                    # scalar engine into bf16 SBUF so the PE can start the next
                    # chunk's accumulation immediately; RoPE runs from SBUF at
                    # DVE 2x/4x bf16 rate.
                    qraw = [stream.tile([128, 512], BF16, tag=f"qraw{s}",
                                        name="qraw") for s in range(2)]
                    kraw = [stream.tile([128, 512], BF16, tag=f"kraw{s}",
                                        name="kraw") for s in range(2)]
                    for sub in range(2):
                        nc.scalar.copy(qraw[sub][:], q_ps[sub][:])
                        nc.scalar.copy(kraw[sub][:], k_ps[sub][:])
                        for ts in range(2):
                            tokt = ch * 4 + sub * 2 + ts
                            src = v_ps[sub][:, ts * 256:(ts + 1) * 256]
                            nc.scalar.copy(
                                v_h[:, tokt, :, 0:64],
                                src.rearrange("p (h c) -> p h c", h=4),
                            )
                    # RoPE: dest[e] = ps[e]*c + ps[o]*(-s); dest[o] = ps[o]*c
                    # + ps[e]*s.  CS = [c,c,c,c]; SN = [+s,-s,+s,-s] per
                    # 32-block. The e<->o partition swap rides on SBUF->SBUF
                    # DMAs (compute engines cannot cross partitions).
                    for src_raw, dst in ((qraw, q_rot), (kraw, k_rot)):
                        for ft in range(2):
                            t1 = stream.tile([128, 512], BF16, tag="t1", name="t1")
                            t2 = stream.tile([128, 512], BF16, tag="t2", name="t2")
                            t2s = stream.tile([128, 512], BF16, tag="t2s", name="t2s")
                            for sub in range(2):
                                sc = slice(sub * 256, (sub + 1) * 256)
                                tc_ = slice(ch * 512 + sub * 256, ch * 512 + (sub + 1) * 256)
                                nc.vector.tensor_mul(
                                    t1[:, sc], src_raw[sub][:, ft * 256:(ft + 1) * 256],
                                    cs_sb[:, tc_],
                                )
                                nc.vector.tensor_mul(
                                    t2[:, sc], src_raw[sub][:, ft * 256:(ft + 1) * 256],
                                    sn_sb[:, tc_],
                                )
                            for hb, eng in ((0, nc.gpsimd), (1, nc.scalar)):
                                e = slice(hb * 64, hb * 64 + 32)
                                o = slice(hb * 64 + 32, hb * 64 + 64)
                                eng.dma_start(out=t2s[e, :], in_=t2[o, :])
                                eng.dma_start(out=t2s[o, :], in_=t2[e, :])
                            nc.vector.tensor_add(dst[:, ft, tok], t1[:], t2s[:])
                for hpi in range(2):
                    nc.sync.dma_start(
                        out=pj_sb[:, hpi, :], in_=projT[hpi * 128:(hpi + 1) * 128, :]
                    )

            # ---- Phase 2-4: causal attention + normalize + projection ----
            # qb-major; the denominator chain for qb and the projection for
            # qb-1 are emitted between attention blocks so they hide under
            # the next qb's (ACT-bound) attention.
            with tc.tile_pool(name=f"aps{rep}", bufs=1, space="PSUM") as aps:

                def emit_attn(hp, qb):
                    hA = 2 * hp
                    hB = 2 * hp + 1
                    nkt = 4 * qb + 4
                    uA = aps.tile([65, 512], F32, tag="uA", name="uA")
                    uB = aps.tile([65, 512], F32, tag="uB", name="uB")

                    def emit_scores(kt):
                        j = kt - 4 * qb
                        off = max(j, 0) * 128
                        ks = slice(kt * 128, (kt + 1) * 128)
                        qs = slice(qb * 512 + off, (qb + 1) * 512)
                        # both heads side by side in one 2-bank psum tile
                        sAB = aps.tile([128, 1024], F32, tag="sAB", name="sAB",
                                       bufs=2)
                        diag = j >= 0
                        nc.tensor.matmul(
                            sAB[:, off:512], k_rot[0:64, hp, ks],
                            q_rot[0:64, hp, qs], start=True, stop=not diag,
                        )
                        nc.tensor.matmul(
                            sAB[:, 512 + off:1024], k_rot[64:128, hp, ks],
                            q_rot[64:128, hp, qs], start=True, stop=not diag,
                        )
                        if diag:
                            # add the causal -inf wedge on the PE: I.T @ mask
                            nc.tensor.matmul(
                                sAB[:, off:off + 128], idb_sb[:], mask_sb[:],
                                start=False, stop=True,
                            )
                            nc.tensor.matmul(
                                sAB[:, 512 + off:512 + off + 128], idb_sb[:],
                                mask_sb[:], start=False, stop=True,
                            )
                        return sAB, off, kt

                    def emit_exp_pv(st):
                        sAB, off, kt = st
                        n = 512 - off
                        pAB = ptiles.tile([128, 1024], BF16, tag="pAB", name="pAB")
                        s_v = sAB[:].rearrange("p (b f) -> p b f", b=2)[:, :, off:512]
                        p_v = pAB[:].rearrange("p (b f) -> p b f", b=2)[:, :, off:512]
                        nc.scalar.activation(p_v, s_v, AF.Exp)
                        nc.tensor.matmul(
                            uA[0:65, off:512],
                            v_sb[:, kt, hA * 65:(hA + 1) * 65],
                            pAB[:, off:512],
                            start=(kt == 0), stop=(kt == nkt - 1),
                        )
                        nc.tensor.matmul(
                            uB[0:65, off:512],
                            v_sb[:, kt, hB * 65:(hB + 1) * 65],
                            pAB[:, 512 + off:1024],
                            start=(kt == 0), stop=(kt == nkt - 1),
                        )

                    prev = emit_scores(0)
                    for kt in range(1, nkt):
                        cur = emit_scores(kt)
                        emit_exp_pv(prev)
                        prev = cur
                    emit_exp_pv(prev)

                    for u_ps, h in ((uA, hA), (uB, hB)):
                        r = h * 4 + qb
                        nc.vector.tensor_copy(u_sb[:, r, :], u_ps[0:65, :])

                def emit_chain_a(qb):
                    # l rows of this qb -> l_sb partitions 32qb..32qb+3,
                    # then transpose to columns and take reciprocals
                    nc.gpsimd.dma_start(
                        out=l_sb[32 * qb:32 * qb + 4, :],
                        in_=u_sb[64:65, :, :].rearrange(
                            "p (h q) f -> p h q f", q=4)[:, :, qb, :],
                    )
                    lt_ps = aps.tile([128, 16], F32, tag="lch", name="lt_ps")
                    for sg in range(4):
                        nc.tensor.matmul(
                            lt_ps[:, sg * 4:(sg + 1) * 4],
                            l_sb[32 * qb:32 * qb + 4, sg * 128:(sg + 1) * 128],
                            id_sb[32 * qb:32 * qb + 4, 32 * qb:32 * qb + 4]
                            if qb < 3 else id_sb[96:100, 96:100],
                            start=True, stop=True, is_transpose=True,
                            tile_position=(32 * qb, 0),
                        )
                    nc.vector.reciprocal(
                        linv_col[:, 16 * qb:16 * qb + 16], lt_ps[:]
                    )
                    return lt_ps

                def emit_chain_b(qb):
                    lvt_ps = aps.tile([16, 128], F32, tag="lch", name="lvt_ps")
                    nc.tensor.matmul(
                        lvt_ps[:], linv_col[:, 16 * qb:16 * qb + 16], id_sb[:],
                        start=True, stop=True, is_transpose=True,
                    )
                    lr = stream.tile([16, 128], F32, tag="lr", name="lr")
                    nc.vector.tensor_copy(lr[:], lvt_ps[:])
                    nc.gpsimd.dma_start(
                        out=linv_dram[16 * qb:16 * qb + 16, :], in_=lr[:]
                    )
                    for h in range(4):
                        hp, hh = h // 2, h % 2
                        r = h * 4 + qb
                        # linv_dram row 16qb + sg*4 + h = 1/l, segment sg
                        src = bass.AP(
                            linv_dram.tensor,
                            (16 * qb + h) * 128,
                            [[0, 64], [512, 4], [1, 128]],
                        )
                        lb = stream.tile([64, 512], F32, tag="lb", name="lb")
                        nc.gpsimd.dma_start(out=lb[:], in_=src)
                        u2t = stream.tile([64, 512], BF16, tag="u2t", name="u2t")
                        nc.vector.tensor_mul(u2t[:], u_sb[0:64, r, :], lb[:])
                        nc.sync.dma_start(
                            out=u2_sb[hh * 64:(hh + 1) * 64, hp,
                                      qb * 512:(qb + 1) * 512],
                            in_=u2t[:],
                        )

                def emit_proj(qb):
                    for m in range(4 * qb, 4 * qb + 4):
                        ms = slice(m * 128, (m + 1) * 128)
                        ob = stream.tile([128, 1024], BF16, tag="ob", name="ob")
                        for nh in range(2):
                            pp = aps.tile([128, 512], F32, tag="pp", name="pp")
                            nc.tensor.matmul(
                                pp[:],
                                u2_sb[:, 0, ms],
                                pj_sb[:, 0, nh * 512:(nh + 1) * 512],
                                start=True, stop=False,
                            )
                            nc.tensor.matmul(
                                pp[:],
                                u2_sb[:, 1, ms],
                                pj_sb[:, 1, nh * 512:(nh + 1) * 512],
                                start=False, stop=True,
                            )
                            nc.vector.tensor_copy(
                                ob[:, nh * 512:(nh + 1) * 512], pp[:]
                            )
                        (nc.sync if m % 2 == 0 else nc.gpsimd).dma_start(
                            out=out[ms, :], in_=ob[:]
                        )

                for qb in range(4):
                    for hp in range(2):
                        emit_attn(hp, qb)
                    emit_chain_a(qb)
                    emit_chain_b(qb)
                    if qb > 0:
                        emit_proj(qb - 1)
                emit_proj(3)
    return nc


def _get_nc(reps=1):
    key = f"nc{reps}"
    if key not in _CACHE:
        import concourse.bacc as bacc

        nc = bacc.Bacc("TRN2", target_bir_lowering=False, debug=False)
        _build_body(nc, reps=reps)
        nc.compile()
        _CACHE[key] = nc
    return _CACHE[key]


def _prep_in_maps(x, freqs_cos, freqs_sin, qkv_w, proj_w):
    x = np.asarray(x, dtype=np.float32)
    cos = np.asarray(freqs_cos, dtype=np.float32)
    sin = np.asarray(freqs_sin, dtype=np.float32)
    qkv_w = np.asarray(qkv_w, dtype=np.float32)
    proj_w = np.asarray(proj_w, dtype=np.float32)

    import ml_dtypes

    sq = np.float32((1.0 / np.sqrt(D)) ** 0.5)
    cosT = np.ascontiguousarray(cos.T) * sq  # (32, T)
    sinT = np.ascontiguousarray(sin.T) * sq
    CS = np.tile(cosT, (4, 1)).astype(ml_dtypes.bfloat16)
    SN = np.tile(np.concatenate([sinT, -sinT], axis=0), (2, 1)).astype(
        ml_dtypes.bfloat16
    )
    f = np.arange(128)
    maskc = np.where(f[None, :] >= f[:, None], 0.0, -1e30).astype(ml_dtypes.bfloat16)
    ident = np.eye(128, dtype=np.float32)
    identb = np.eye(128, dtype=ml_dtypes.bfloat16)
    perm = np.concatenate([np.arange(0, D, 2), np.arange(1, D, 2)])

    in_maps = []
    for core in range(NCORES):
        b = core // 4
        g = core % 4
        heads = [4 * g + j for j in range(HPC)]
        q_rows = np.concatenate([h * D + perm for h in heads])
        k_rows = np.concatenate([C + h * D + perm for h in heads])
        v_rows = np.concatenate([2 * C + h * D + np.arange(D) for h in heads])
        wTc = np.ascontiguousarray(
            np.concatenate(
                [qkv_w[q_rows, :], qkv_w[k_rows, :], qkv_w[v_rows, :]], axis=0
            ).T
        ).astype(ml_dtypes.bfloat16)  # (1024, 768)
        vcols = np.concatenate([h * D + np.arange(D) for h in heads])

        projTc = np.ascontiguousarray(proj_w[:, vcols].T).astype(ml_dtypes.bfloat16)
        xTc = np.ascontiguousarray(x[b].T).astype(ml_dtypes.bfloat16)  # (1024, 2048)
        in_maps.append(
            {
                "xT": xTc,
                "wT": wTc,
                "projT": projTc,
                "CS": CS,
                "SN": SN,
                "maskc": maskc,
                "identb": identb,
                "ident": ident,
            }
        )
    return in_maps


def _get_runner(reps=1):
    """Build (once) a jitted 8-core shard_map executable mirroring
    bass2jax.run_bass_via_pjrt, without donation so it can be re-run for
    timing with device-resident inputs."""
    rkey = f"runner{reps}"
    if rkey in _CACHE:
        return _CACHE[rkey]
    import jax
    import concourse.mybir as mybir
    from concourse import bass2jax
    from jax.experimental.shard_map import shard_map
    from jax.sharding import Mesh, PartitionSpec

    nc = _get_nc(reps)
    bass2jax.install_neuronx_cc_hook()

    in_names = []
    out_names = []
    out_avals = []
    zero_outs = []
    pname = nc.partition_id_tensor.name if nc.partition_id_tensor else None
    for alloc in nc.m.functions[0].allocations:
        if not isinstance(alloc, mybir.MemoryLocationSet):
            continue
        name = alloc.memorylocations[0].name
        if alloc.kind == "ExternalInput":
            if name != pname:
                in_names.append(name)
        elif alloc.kind == "ExternalOutput":
            shape = tuple(alloc.tensor_shape)
            dtype = mybir.dt.np(alloc.dtype)
            out_names.append(name)
            out_avals.append(jax.core.ShapedArray(shape, dtype))
            zero_outs.append(np.zeros(shape, dtype))
    n_params = len(in_names)
    all_names = list(in_names) + list(out_names)
    if pname is not None:
        all_names.append(pname)

    def _body(*args):
        operands = list(args)
        if pname is not None:
            operands.append(bass2jax.partition_id_tensor())
        outs = bass2jax._bass_exec_p.bind(
            *operands,
            out_avals=tuple(out_avals),
            in_names=tuple(all_names),
            out_names=tuple(out_names),
            lowering_input_output_aliases=(),
            sim_require_finite=True,
            sim_require_nnan=True,
            nc=nc,
        )
        return tuple(outs)

    devices = jax.devices()[:NCORES]
    mesh = Mesh(np.asarray(devices), ("core",))
    nin = n_params + len(out_names)
    sharded_body = shard_map(
        _body,
        mesh=mesh,
        in_specs=(PartitionSpec("core"),) * nin,
        out_specs=(PartitionSpec("core"),) * len(out_names),
        check_rep=False,
    )
    sharded = jax.jit(sharded_body, keep_unused=True)
    _CACHE[rkey] = (sharded, in_names, out_names, zero_outs, mesh)
    return _CACHE[rkey]


def kernel(x, freqs_cos, freqs_sin, qkv_w, proj_w):
    import jax
    from jax.sharding import NamedSharding, PartitionSpec

    global LAST_EXEC_NS, LAST_RESULTS
    sharded, in_names, out_names, zero_outs, mesh = _get_runner()
    in_maps = _prep_in_maps(x, freqs_cos, freqs_sin, qkv_w, proj_w)

    concat_in = [
        np.concatenate([in_maps[c][n] for c in range(NCORES)], axis=0)
        for n in in_names
    ]
    concat_zero = [
        np.zeros((NCORES * z.shape[0], *z.shape[1:]), z.dtype) for z in zero_outs
    ]
    sharding = NamedSharding(mesh, PartitionSpec("core"))
    dev_args = [jax.device_put(a, sharding) for a in concat_in + concat_zero]

    out_arrs = sharded(*dev_args)
    jax.block_until_ready(out_arrs)

    iters = int(os.environ.get("KERNEL_TIME_ITERS", "0"))
    if iters > 0:
        import time

        # Differential timing: one NEFF with the body repeated 8x vs 1x;
        # paired alternating rounds cancel dispatch overhead and drift.
        sharded8 = _get_runner(reps=8)[0]
        jax.block_until_ready(sharded8(*dev_args))

        def one_round(fn):
            t0 = time.monotonic()
            for _ in range(iters):
                r = fn(*dev_args)
            jax.block_until_ready(r)
            return (time.monotonic() - t0) / iters

        diffs = []
        for _ in range(6):
            t1 = one_round(sharded)
            t8 = one_round(sharded8)
            diffs.append((t8 - t1) / 7 * 1e9)
        diffs.sort()
        LAST_EXEC_NS = diffs[len(diffs) // 2]
        _CACHE["exec_ns_min"] = diffs[0]

    out = np.asarray(out_arrs[out_names.index("out")]).astype(np.float32)
    out = out.reshape(NCORES, T, C)
    return np.stack(
        [
            out[0] + out[1] + out[2] + out[3],
            out[4] + out[5] + out[6] + out[7],
        ],
        axis=0,
    )

